# revision 25
# baseline (speedup 1.0000x reference)
"""Multi-head self-attention (RoPE, causal) Trainium2 Bass kernel.

Full inputs in, full output out. Internally shards across 8 NeuronCores:
data-parallel over batch (2) x tensor-parallel over heads (16 -> 4 per core).
Each core computes its 4 heads' attention and the partial WO contraction;
the host sums the 4 partials per batch (the "all-reduce" of the sharding
hint, done on host since outputs are gathered anyway).

Device layouts (per core, heads h = 0..3 local):
  xT    [1024, 2048]  x[b].T (bf16)
  wq/wk [1024, 256]   W shard transposed, columns permuted per head:
                      head h occupies rows 64h..64h+63 of the projection
                      output, rows 0:32 = even dims, 32:64 = odd dims.
                      Projection m-tile j covers heads {2j, 2j+1}.
  RoPE: t1 = raw*cos, t2 = raw*sin (full-width DVE), then 4 quarter-height
        combines with +-32-partition-offset operands produce rotated
        QC/KC [128, 2048] tiles directly in the per-head [E;O] layout.
  scores^T: per head ONE matmul per key tile, contraction 64
            (rows 64m:64m+64 of QC/KC tile j), output [128 keys, q].
  V_sb  [128, 16*4*65] V in [key, (kt, h, dk+1)] with a trailing ones column
                      per head -> MM2's 65th output row is the softmax
                      denominator.
  exp on ACT with fused 1/sqrt(dk) scale; diagonal-block tiles get a
  rectangle restriction plus a triangular 0/1 mask after exp (Pool).
  PSUM->SBUF staging copies ride the Pool engine; DVE keeps RoPE and
  normalization multiplies. Output DMA is one descriptor per token chunk.
  Matmul operands are bf16 (fp32 PSUM accum; rel err ~5e-3).
"""

from contextlib import ExitStack

import numpy as np

import concourse.bass as bass
import concourse.tile as tile
from concourse import bacc, library_config, mybir
from concourse.bass_utils import run_bass_kernel_spmd
from concourse._compat import with_exitstack

F32 = mybir.dt.float32
F32R = mybir.dt.float32r
BF16 = mybir.dt.bfloat16
MM_DT = BF16
AF = mybir.ActivationFunctionType
ALU = mybir.AluOpType

B = 2
SEQ = 2048
DM = 1024
H = 16
DK = 64
THETA = 10000.0

N_CORES = 8
HPC = 4           # heads per core
TCH = 512         # token chunk
NTC = SEQ // TCH  # 4
KD = DM // 128    # 8 contraction chunks
NKT = SEQ // 128  # 16 key tiles


def _wo_block(nc, ps, outsb, attnT, wo_sb, out, t):
    """Output projection + writeback for token chunk t (one DMA)."""
    o_sb = outsb.tile([128, 4096], F32, tag="osb", name=f"osb_{t}")
    for tt in range(4):
        tg = 4 * t + tt
        for d2 in range(2):
            o_ps = ps.tile([128, 512], F32, tag="proj", bufs=2,
                           name=f"ops_{t}_{tt}_{d2}")
            for r in range(2):
                nc.tensor.matmul(
                    o_ps[:],
                    lhsT=attnT[r][:, 128 * tg:128 * tg + 128],
                    rhs=wo_sb[:, 1024 * r + 512 * d2:1024 * r + 512 * d2 + 512],
                    start=(r == 0), stop=(r == 1))
            nc.vector.tensor_copy(
                o_sb[:, 1024 * tt + 512 * d2:1024 * tt + 512 * d2 + 512],
                o_ps[:])
    nc.sync.dma_start(
        out[512 * t:512 * t + 512, :].rearrange("(tt p) d -> p tt d", p=128),
        o_sb.rearrange("p (tt d) -> p tt d", tt=4))


@with_exitstack
def _mha_body(ctx: ExitStack, tc_: tile.TileContext, aps, repeat=1):
    nc = tc_.nc
    xT, wq, wk, wv, wo, cosT, sinT, mask01, out = aps

    const = ctx.enter_context(tc_.tile_pool(name="const", bufs=1))
    xkp = ctx.enter_context(tc_.tile_pool(name="xkp", bufs=2))
    work = ctx.enter_context(tc_.tile_pool(name="work", bufs=4))
    rawp = ctx.enter_context(tc_.tile_pool(name="rawp", bufs=3))
    prb = ctx.enter_context(tc_.tile_pool(name="prb", bufs=8))
    nrm = ctx.enter_context(tc_.tile_pool(name="nrm", bufs=4))
    outsb = ctx.enter_context(tc_.tile_pool(name="outsb", bufs=2))
    ps = ctx.enter_context(tc_.tile_pool(name="ps", bufs=1, space="PSUM"))

    nc.gpsimd.load_library(library_config.proxy)

    # ---- constants / weights to SBUF
    wq_sb = const.tile([128, 2048], MM_DT, name="wq_sb")
    nc.sync.dma_start(wq_sb.rearrange("p (k m) -> p k m", k=KD),
                      wq.rearrange("(k p) m -> p k m", p=128))
    wk_sb = const.tile([128, 2048], MM_DT, name="wk_sb")
    nc.sync.dma_start(wk_sb.rearrange("p (k m) -> p k m", k=KD),
                      wk.rearrange("(k p) m -> p k m", p=128))
    wv_sb = const.tile([128, 2048], MM_DT, name="wv_sb")
    nc.sync.dma_start(wv_sb.rearrange("p (k m) -> p k m", k=KD),
                      wv.rearrange("(k p) m -> p k m", p=128))
    wo_sb = const.tile([128, 2048], MM_DT, name="wo_sb")
    nc.sync.dma_start(wo_sb.rearrange("p (r d) -> p r d", r=2),
                      wo.rearrange("(r p) d -> p r d", p=128))
    cos_sb = const.tile([128, 2048], BF16, name="cos_sb")
    nc.sync.dma_start(cos_sb[:], cosT[:])
    sin_sb = const.tile([128, 2048], BF16, name="sin_sb")
    nc.sync.dma_start(sin_sb[:], sinT[:])
    mask_sb = const.tile([128, 128], MM_DT, name="mask_sb")
    nc.sync.dma_start(mask_sb[:], mask01[:])

    # persistent per-core state: QC/KC tile j holds heads {2j, 2j+1}, rows
    # 64m..64m+63 = head 2j+m ([0:32]=rotated evens, [32:64]=rotated odds).
    # Two alternating state sets when the timing repeat loop is active, so
    # consecutive iterations can overlap (iteration i+1's projections need
    # not wait for iteration i's last attention reads).
    nsets = 1 if repeat == 1 else 2
    state = []
    for s in range(nsets):
        QCs = [const.tile([128, 2048], MM_DT, name=f"QC{m}_{s}") for m in range(2)]
        KCs = [const.tile([128, 2048], MM_DT, name=f"KC{m}_{s}") for m in range(2)]
        aTs = [const.tile([128, 2048], MM_DT, name=f"attnT{m}_{s}")
               for m in range(2)]
        V_sb = const.tile([128, NKT * HPC * 65], MM_DT, name=f"V_sb_{s}")
        vv = V_sb.rearrange("p (g h m) -> p g h m", h=HPC, m=65)
        nc.scalar.activation(vv[:, :, :, 64],
                             mask_sb[:, 0:64].rearrange("p (g h) -> p g h", h=HPC),
                             AF.Copy, bias=1.0, scale=0.0)
        state.append((QCs, KCs, aTs, vv))

    def iter_body(s):
        QC, KC, attnT, v_view = state[s]
        attnT_prev = state[(s + nsets - 1) % nsets][2]
        _chunks(nc, ps, outsb, xkp, work, rawp, prb, nrm,
                QC, KC, attnT, attnT_prev, v_view,
                xT, wq_sb, wk_sb, wv_sb, wo_sb, cos_sb, sin_sb, mask_sb,
                out, repeat)

    if repeat == 1:
        iter_body(0)
        _wo_block(nc, ps, outsb, state[0][2], wo_sb, out, NTC - 1)
        return
    with tc_.For_i(0, repeat // 2):
        iter_body(0)
        iter_body(1)
    if repeat % 2:
        iter_body(0)


def _chunks(nc, ps, outsb, xkp, work, rawp, prb, nrm,
            QC, KC, attnT, attnT_prev, v_view,
            xT, wq_sb, wk_sb, wv_sb, wo_sb, cos_sb, sin_sb, mask_sb,
            out, repeat):
    for t in range(NTC):
        ts0 = TCH * t
        # ---- load x^T for this token range (one DMA, all 8 d-chunks)
        xk_t = xkp.tile([128, KD * TCH], MM_DT, name=f"xk_{t}", tag="xk", bufs=2)
        nc.sync.dma_start(
            xk_t.rearrange("p (k c) -> p k c", k=KD),
            xT[:, ts0:ts0 + TCH].rearrange("(k p) c -> p k c", p=128))
        xks = [xk_t[:, TCH * k:TCH * k + TCH] for k in range(KD)]

        # ---- V projection (each psum tile covers 2 key-tiles)
        for vp in range(2):
            v_ps = ps.tile([128, 512], F32, tag="proj", bufs=2, name=f"vps_{t}_{vp}")
            for half in range(2):
                lkt = 2 * vp + half
                for k in range(KD):
                    nc.tensor.matmul(
                        v_ps[:, 256 * half:256 * half + 256],
                        lhsT=xks[k][:, 128 * lkt:128 * lkt + 128],
                        rhs=wv_sb[:, 256 * k:256 * k + 256],
                        start=(k == 0), stop=(k == KD - 1),
                        skip_group_check=True)
            gkt = 4 * t + 2 * vp
            nc.vector.tensor_copy(
                v_view[:, gkt:gkt + 2, :, 0:64],
                v_ps.rearrange("p (x h m) -> p x h m", x=2, h=HPC))

        # ---- Q/K projections + RoPE (combined per-head [E;O] layout)
        # m-tile 0 (heads 0,1) for both Q and K first, so the pair-0
        # attention loop can start before pair 1 is projected.
        for m in range(2):
            for w_sb, dstT, wtag in ((wq_sb, QC, "q"), (wk_sb, KC, "k")):
                q_ps = ps.tile([128, 512], F32, tag="proj", bufs=2,
                               name=f"qps_{t}_{m}_{wtag}")
                for k in range(KD):
                    nc.tensor.matmul(
                        q_ps[:],
                        lhsT=w_sb[:, 256 * k + 128 * m:256 * k + 128 * m + 128],
                        rhs=xks[k][:],
                        start=(k == 0), stop=(k == KD - 1))
                raw = rawp.tile([128, 512], MM_DT, tag="raw",
                                name=f"raw_{t}_{m}_{wtag}")
                nc.vector.tensor_copy(raw[:], q_ps[:])
                cs, sn = cos_sb[:, ts0:ts0 + TCH], sin_sb[:, ts0:ts0 + TCH]
                t1 = work.tile([128, 512], MM_DT, tag="ro", name="t1")
                nc.vector.tensor_mul(t1[:], raw[:], cs)
                t2 = work.tile([128, 512], MM_DT, tag="ro", name="t2")
                nc.vector.tensor_mul(t2[:], raw[:], sn)
                # swap the 16-row halves of every 32-partition quadrant:
                # brings the E/O partner (signed via sin_sb) into place
                t2s = work.tile([128, 512], MM_DT, tag="ro", name="t2s")
                nc.vector.stream_shuffle(
                    t2s[:], t2[:], list(range(16, 32)) + list(range(16)))
                nc.gpsimd.tensor_add(dstT[m][:, ts0:ts0 + TCH], t1[:], t2s[:])

        # ---- output projection for the PREVIOUS chunk: PE filler work
        # during this chunk's ACT-bound attention loop
        if t >= 1:
            _wo_block(nc, ps, outsb, attnT, wo_sb, out, t - 1)
        elif repeat > 1:
            _wo_block(nc, ps, outsb, attnT_prev, wo_sb, out, NTC - 1)

        # ---- attention: head pairs, key-tile loop (att = 2 PSUM banks,
        # sc gets 4 for the exp pipeline, proj 2)
        nkt_t = 4 * t + 4
        for pr in range(2):
            att = [ps.tile([128, 512], F32, tag="att", bufs=2,
                           name=f"att_{t}_{pr}_{h2}") for h2 in range(2)]
            for kt in range(nkt_t):
                i = kt - 4 * t  # >= 0 on diagonal-block tiles
                c0 = 128 * i if i >= 0 else 0
                sc = ps.tile([128, 1024], F32, tag="sc", bufs=2,
                             name=f"sc_{t}_{kt}_{pr}")
                scv = sc.rearrange("p (h2 c) -> p h2 c", h2=2)
                for h2 in range(2):
                    nc.tensor.matmul(
                        scv[:, h2, c0:TCH],
                        lhsT=KC[pr][64 * h2:64 * h2 + 64, 128 * kt:128 * kt + 128],
                        rhs=QC[pr][64 * h2:64 * h2 + 64, ts0 + c0:ts0 + TCH],
                        start=True, stop=True)
                pt = prb.tile([128, 1024], MM_DT, tag="pt",
                              name=f"pt_{t}_{kt}_{pr}")
                ptv = pt.rearrange("p (h2 c) -> p h2 c", h2=2)
                nc.scalar.activation(ptv[:, :, c0:TCH], scv[:, :, c0:TCH],
                                     AF.Exp, scale=0.125)
                for h2 in range(2):
                    h = 2 * pr + h2
                    if i >= 0:
                        nc.gpsimd.tensor_mul(ptv[:, h2, c0:c0 + 128],
                                             ptv[:, h2, c0:c0 + 128], mask_sb[:])
                    nc.tensor.matmul(
                        att[h2][0:65, c0:TCH],
                        lhsT=v_view[:, kt, h, :],
                        rhs=ptv[:, h2, c0:TCH],
                        start=(kt == 0), stop=(kt == nkt_t - 1),
                        skip_group_check=True)
            # ---- normalize rows 0:64 by row 64, write into attnT
            for h2 in range(2):
                dr = nrm.tile([1, 512], F32, tag="dr", name=f"dr_{t}_{pr}_{h2}")
                nc.vector.tensor_copy(dr[:], att[h2][64:65, :])
                rr = nrm.tile([1, 512], F32, tag="dr", name=f"rr_{t}_{pr}_{h2}")
                nc.vector.reciprocal_approx_fast(rr[:], dr[:])
                rec = nrm.tile([64, 512], F32, tag="den", name=f"rec_{t}_{pr}_{h2}")
                nc.gpsimd.partition_broadcast(rec[:], rr[:])
                if h2 == 0:
                    nc.vector.tensor_mul(attnT[pr][0:64, ts0:ts0 + TCH],
                                         att[h2][0:64, :], rec[:])
                else:
                    tmp = nrm.tile([64, 512], MM_DT, tag="den",
                                   name=f"tmp_{t}_{pr}_{h2}")
                    nc.vector.tensor_mul(tmp[:], att[h2][0:64, :], rec[:])
                    nc.vector.tensor_copy(attnT[pr][64:128, ts0:ts0 + TCH],
                                          tmp[:])

    if repeat == 1:
        _wo_block(nc, ps, outsb, attnT, wo_sb, out, NTC - 1)


def build_nc(repeat=1):
    nc = bacc.Bacc("TRN2", target_bir_lowering=False, debug=False,
                   enable_asserts=False, num_devices=N_CORES)
    aps = [
        nc.dram_tensor("xT", [DM, SEQ], MM_DT, kind="ExternalInput").ap(),
        nc.dram_tensor("wq", [DM, 256], MM_DT, kind="ExternalInput").ap(),
        nc.dram_tensor("wk", [DM, 256], MM_DT, kind="ExternalInput").ap(),
        nc.dram_tensor("wv", [DM, 256], MM_DT, kind="ExternalInput").ap(),
        nc.dram_tensor("wo", [256, DM], MM_DT, kind="ExternalInput").ap(),
        nc.dram_tensor("cosT", [128, SEQ], BF16, kind="ExternalInput").ap(),
        nc.dram_tensor("sinT", [128, SEQ], BF16, kind="ExternalInput").ap(),
        nc.dram_tensor("mask01", [128, 128], MM_DT, kind="ExternalInput").ap(),
        nc.dram_tensor("out", [SEQ, DM], F32, kind="ExternalOutput").ap(),
    ]
    with tile.TileContext(nc) as t:
        _mha_body(t, aps, repeat=repeat)
    nc.compile()
    return nc


_NC = {}


def _get_nc(repeat=1):
    if repeat not in _NC:
        _NC[repeat] = build_nc(repeat)
    return _NC[repeat]


def _qk_perm():
    """Column permutation for one 256-row W shard: head h (0..3) occupies
    output rows 64h..64h+63. Within a head, quadrant-friendly layout:
    row 32a + 16e + i (a = freq group, e = even/odd, i = freq-in-group)
    <- shard row 64h + 2*(16a+i) + e, so the RoPE partner of any row is a
    16-row swap inside the same 32-partition quadrant (stream_shuffle)."""
    perm = []
    for h in range(HPC):
        for a in range(2):
            for e in range(2):
                for i in range(16):
                    perm.append(64 * h + 2 * (16 * a + i) + e)
    return np.array(perm, dtype=np.int64)


def make_in_maps(x, token_positions, WQ, WK, WV, WO):
    np_mm = mybir.dt.np(MM_DT)
    x = np.asarray(x, dtype=np.float32)
    WQ, WK, WV, WO = (np.asarray(w, dtype=np.float32) for w in (WQ, WK, WV, WO))
    pos = np.asarray(token_positions).astype(np.float64)

    half = DK // 2
    inv = 1.0 / (THETA ** (2.0 * np.arange(half, dtype=np.float64) / DK))
    freqs = pos[:, None] * inv[None, :]                      # [SEQ, 32]
    import ml_dtypes
    _ct = ml_dtypes.bfloat16
    cos_f, sin_f = np.cos(freqs).T, np.sin(freqs).T          # [32, SEQ]
    # row 32a+16e+i (mod 64) uses freq 16a+i; sin sign: + on e=0 rows
    # (their product lands on the odd-dim output), - on e=1 rows
    cosT = np.empty((128, SEQ), dtype=np.float64)
    sinT = np.empty((128, SEQ), dtype=np.float64)
    for p in range(128):
        a = (p % 64) // 32
        e = (p % 32) // 16
        i = p % 16
        f = 16 * a + i
        cosT[p] = cos_f[f]
        sinT[p] = sin_f[f] if e == 0 else -sin_f[f]
    cosT = np.ascontiguousarray(cosT.astype(_ct))
    sinT = np.ascontiguousarray(sinT.astype(_ct))
    mask01 = np.triu(np.ones((128, 128), dtype=np_mm))

    perm = _qk_perm()
    xTs = [np.ascontiguousarray(x[b].T) for b in range(B)]
    in_maps = []
    for c in range(N_CORES):
        b, g = divmod(c, N_CORES // B)
        sl = slice(256 * g, 256 * (g + 1))
        in_maps.append({
            "xT": xTs[b].astype(np_mm),
            "wq": np.ascontiguousarray(WQ[sl, :][perm, :].T).astype(np_mm),
            "wk": np.ascontiguousarray(WK[sl, :][perm, :].T).astype(np_mm),
            "wv": np.ascontiguousarray(WV[sl, :].T).astype(np_mm),
            "wo": np.ascontiguousarray(WO[:, sl].T).astype(np_mm),
            "cosT": cosT,
            "sinT": sinT,
            "mask01": mask01,
        })
    return in_maps


def run(in_maps, trace=False, **kw):
    nc = _get_nc()
    return run_bass_kernel_spmd(nc, in_maps, list(range(N_CORES)), trace=trace, **kw)


def kernel(x, token_positions, WQ, WK, WV, WO):
    in_maps = make_in_maps(x, token_positions, WQ, WK, WV, WO)
    res = run(in_maps)
    out = np.zeros((B, SEQ, DM), dtype=np.float32)
    for c in range(N_CORES):
        out[c // (N_CORES // B)] += res.results[c]["out"]
    return out
